# revision 37
# baseline (speedup 1.0000x reference)
"""Self-attention kernel for Trainium2, SPMD across 8 NeuronCores.

Reference computation (fp32):
    q = x @ Wq + bq; k = x @ Wk + bk; v = x @ Wv + bv
    out = softmax((q @ k.T) / sqrt(d_q), axis=1) @ v

Sharding: rows of Q (sequence dim N=8192) are sharded across the 8 cores
(1024 rows each).  K/V are computed redundantly on every core — a measured
ncfw AllGather of K/V on this chip costs ~160us, far more than the ~59us
of redundant projection matmuls.

Host-side layout: x.T is pre-arranged into 16 token-blocks of shape
[128, 4096] (bfloat16) where each partition row is contiguous in DRAM
(one 8KB DMA descriptor per partition).  The block axis is rotated per
core so block 0 holds the core's own Q tokens; the attention j-loop
order does not affect the softmax sums.  Weights are packed (Wk|Wv|Wq)
so the 0.5MB the first projection needs lands first.

Precision: Q/x/weights bfloat16; the V/exp side (V tiles, exp outputs,
denominator accumulators) float16 (2x DVE throughput, 10-bit mantissa);
the K PROJECTION of blocks 4-15 runs in fp8-e4m3 DoubleRow (2 MACs/
cycle on the PE, halving that part of the 55us replicated-projection
cost).  Blocks 0-3 project K from the bf16 x instead: the early DMA
ramp cannot feed the extra fp8 stream and the PE would idle ~10us
waiting for it (measured; this single change took 155.6us -> 145.5us).
Wk and bk are host-prescaled by 16 on BOTH paths (keeps e4m3 in its
normal range); kT_sb holds 16*K everywhere and the 1/16 folds into the
exp scale for zero extra ops.  Only K can take fp8: its noise enters
through softmax scores only (measured rel err 6.4e-3 vs the 2e-2 gate),
while fp8 V or Q noise would hit the output directly (~5%/2.4%, both
over the gate).  All matmuls accumulate in fp32 PSUM.  PSUM matmuls are
512 wide (bank-boundary limit).

Per-core dataflow, streamed block by block with the attention one block
behind the projection stream:
  - ~4us of dummy matmuls during the initial DMA wait pre-warm the PE
    HAM clock gate to 2.4GHz
  - K^T[dk, 8192], V^T -> V[j, dv] (PE transpose), Q^T[dq, 1024 local];
    biases added during the DVE PSUM->SBUF eviction
  - per j-tile (128 keys): S^T[kj, qi] = K_tile^T.T @ Q^T (two query
    halves into one 2-bank PSUM tile); one [128,1024] exp on ACT
    (scale=1/sqrt(128), no max subtraction needed -- |scores| < ~3);
    softmax denominator accumulated on DVE; O^T[dv, qi] += V_tile.T @ E
    accumulated in PSUM across all 64 j-tiles.  The V matmuls run TWO
    j-tiles behind the S matmuls (software pipeline): lag-1 left the
    in-order PE eating one exp latency per tile at the stream tail and
    micro-stalls mid-stream; lag-2 measured 145.5us -> 139.8us.
  - denominator: two fp16 accumulators (j-tiles 0-31 / 32-63); the first
    half's partition-sum ones-matmuls run mid-stream (after block 9) so
    only the second half sits on the serial tail.
  - epilogue: remaining ones-matmuls, DVE reciprocal, O^T copied out of
    PSUM on the (tail-idle) scalar engine, PE transpose, 1/den scale on
    DVE, store.  Timeline-verified: compute ends ~152us with no PE gap
    >0.4us between t=26us and t=143us; the last ~10us is the Tile drain/
    reset exit sequence (framework, not kernel work).

Engine balance (2-run cluster 139.8/140.4us): PE ~122us busy -- the
bottleneck -- gap-free mid-stream; ~24MB DMA (bf16 x 16MB +
fp8 x for blocks 4-15 + weights); ACT ~73us (exp, the only exp engine);
DVE ~81us (denominator accumulation + evictions).  Remaining non-PE
wall time: ~3us framework preamble (gpsimd boot barrier), ~5us start
ramp, ~3.9us attention-end pipeline drain (serial exp(63)->acc->den-B
chain), ~6us Tile drain/reset exit.
A 10-deep exp pool decouples ACT's exp stream from DVE/PE consumption
bursts (6-deep measurably stalls the PE).  Fixed overheads: ~7us
framework preamble, ~8us tail drain barrier.

Rejected roads (measured in this environment): sharding the K/V
projections needs a cross-core exchange -- the ncfw AllGather has ~85us
FIXED cost (measured: 88us for 32KB, 106us for 0.5MB, incl ~15us kernel
overhead), and raw SBUF->SBUF remote_dma_broadcast works only between
same-SEngine core pairs here (cross-SE transfers wedge the SDMA engines
until a device reset).  Full fp8 fails accuracy (e4m3's ~3.6% element
noise puts ~5% on q/k/v and ~2.5-6% on the output, over the 2e-2 gate);
a K-only fp8 DoubleRow projection (Wk,bk host-prescaled by 16, the 1/16
folded into the exp scale) is accuracy-viable (rel err 8.0e-3) and cuts
PE busy to ~123.5us, but its +8MB x copy pushes DMA to ~93us and wall
time did not reliably improve (155.3us then 182.9us across runs);
casting x to fp8 on-chip instead regressed to 185us (DVE lands in the
per-block projection critical path).  The bf16 kernel's 4-run cluster
is 155.7-158.4us.
"""

import numpy as np

import concourse.bacc as bacc
import concourse.mybir as mybir
import concourse.tile as tile
from concourse.bass_utils import run_bass_kernel_spmd
from concourse.masks import make_identity

N_CORES = 8
N = 8192          # sequence length
D = 1024          # d_model
DH = 128          # d_q == d_k == d_v
NB = N // N_CORES # tokens per core (1024)
KT = D // 128     # k-tiles in the contraction over d_model (8)
JBLK = 512        # token block for the K/V projection stream
NJB = N // JBLK   # 16
NJT = N // 128    # 64 j-tiles in the attention loop
QBLK = 512        # query block (fp32 moving-operand max)
NQB = NB // QBLK  # 2
FB = KT * JBLK    # 4096 floats per partition per stream block

F32 = mybir.dt.float32
BF16 = mybir.dt.bfloat16
FP16 = mybir.dt.float16
FP8 = mybir.dt.float8e4
SCALE = 1.0 / float(np.sqrt(DH))
# K path: Wk,bk are host-prescaled by 16 (keeps e4m3 in its normal range);
# kT_sb holds 16*K and the 1/16 folds into the exp scale.
SCALE_K = SCALE / 16.0

_CACHE = {}

# Results of the last run_bass_kernel_spmd call (for the test harness to
# read exec_time_ns etc. when tracing is enabled via BASS_TRACE).
LAST_RESULTS = None


def _emit(ctx, tc, nc, xT, xT8, wk8, w_all, b_all, out):
    singles = ctx.enter_context(tc.tile_pool(name="singles", bufs=1))
    xt_pool = ctx.enter_context(tc.tile_pool(name="xt", bufs=7))
    x8_pool = ctx.enter_context(tc.tile_pool(name="x8", bufs=4))
    vt_pool = ctx.enter_context(tc.tile_pool(name="vt", bufs=3))
    exp_pool = ctx.enter_context(tc.tile_pool(name="exp", bufs=10))
    oT_pool = ctx.enter_context(tc.tile_pool(name="oT", bufs=3))
    o_pool = ctx.enter_context(tc.tile_pool(name="o", bufs=3))
    ps_pool = ctx.enter_context(tc.tile_pool(name="ps", bufs=2, space="PSUM"))
    pp_pool = ctx.enter_context(tc.tile_pool(name="pp", bufs=2, space="PSUM"))
    po_pool = ctx.enter_context(tc.tile_pool(name="po", bufs=1, space="PSUM"))

    # --- constants / weights ---------------------------------------------
    # w_all layout is (Wk | Wv | Wq); K+V land first in a 0.5MB DMA so the
    # first stream block's projections start as early as possible.
    w_sb = singles.tile([128, 3 * D], BF16, tag="w_sb")
    nc.sync.dma_start(out=w_sb[:, 0:2 * D], in_=w_all[:, 0:2 * D])
    nc.sync.dma_start(out=w_sb[:, 2 * D:3 * D], in_=w_all[:, 2 * D:3 * D])
    b_sb = singles.tile([128, 3], F32, tag="b_sb")
    nc.sync.dma_start(out=b_sb, in_=b_all)
    wk8_sb = singles.tile([128, 4, 2, 128], FP8, tag="wk8")
    nc.sync.dma_start(out=wk8_sb, in_=wk8)
    ident_bf = singles.tile([128, 128], BF16, tag="ident_bf")
    ident_f16 = singles.tile([128, 128], FP16, tag="ident_f16")
    ones128 = singles.tile([128, 1], FP16, tag="ones128")
    nc.vector.memset(ones128, 1.0)

    W_BASE = {1: 0, 2: D, 0: 2 * D}  # k, v, q order in w_all

    def w_ap(proj, kt):  # lhsT [128, 128] for projection matmuls
        base = W_BASE[proj] + kt * 128
        return w_sb[:, base:base + 128]

    # --- persistent SBUF tensors -----------------------------------------
    kT_sb = singles.tile([128, N], BF16, tag="kT")    # K^T, all tokens
    v_sb = singles.tile([128, N], FP16, tag="v")      # V natural, 64 j-tiles
    qT_sb = singles.tile([128, NB], BF16, tag="qT")   # Q^T, local tokens
    rden_sb = singles.tile([128, NB // 128], F32, tag="rden")
    acc_a = singles.tile([128, NB], FP16, tag="acc_a", name="acc_a")
    acc_b = singles.tile([128, NB], FP16, tag="acc_b", name="acc_b")
    den_ab = singles.tile([128, NB // 128], F32, tag="den_ab")
    po_t = po_pool.tile([128, NB], F32, tag="po", name="po_t")

    def stream_block(jb):
        """DMA block jb and project its K^T / V columns (+ Q^T for jb<2).

        Block 0 is fetched as two half tiles so the first projection
        matmuls can start after 1MB instead of 2MB of transfer."""
        if jb == 0:
            ha = xt_pool.tile([128, FB // 2], BF16, tag="xt", name="xt0a")
            nc.gpsimd.dma_start(out=ha, in_=xT[0, :, 0:FB // 2])
            hb = xt_pool.tile([128, FB // 2], BF16, tag="xt", name="xt0b")
            nc.gpsimd.dma_start(out=hb, in_=xT[0, :, FB // 2:FB])
            # identities built here: after block 0's DMA issues (so they
            # don't delay them on gpsimd) but before any transpose reads
            make_identity(nc, ident_bf)
            make_identity(nc, ident_f16)
            parts = ((ha, 0), (hb, KT // 2))
        else:
            xt_t = xt_pool.tile([128, FB], BF16, tag="xt", name=f"xt{jb}")
            nc.gpsimd.dma_start(out=xt_t, in_=xT[jb])
            parts = ((xt_t, 0),)
        if jb >= 4:
            x8_t = x8_pool.tile([128, KT, JBLK], FP8, tag="x8",
                                name=f"x8{jb}")
            nc.sync.dma_start(out=x8_t, in_=xT8[jb])

        def xsl(kt):
            for t, base in reversed(parts):
                if kt >= base:
                    return t[:, (kt - base) * JBLK:(kt - base + 1) * JBLK]

        tok = slice(jb * JBLK, (jb + 1) * JBLK)

        # K projection.  Blocks 0-3 use the bf16 x that is already loaded
        # (w_all's K section is host-prescaled by 16 to match): the early
        # DMA ramp cannot feed the extra fp8 stream and the PE would idle
        # ~10us waiting for it.  From block 4 on, fp8 DoubleRow (2 MACs/
        # cycle) halves the projection cost; both paths land 16*K in kT_sb.
        ps_k = pp_pool.tile([128, JBLK], F32, tag="pp")
        if jb < 4:
            for kt in range(KT):
                nc.tensor.matmul(ps_k, w_ap(1, kt), xsl(kt),
                                 start=(kt == 0), stop=(kt == KT - 1))
        else:
            for g in range(JBLK // 256):
                gs = slice(g * 256, (g + 1) * 256)
                for s in range(4):
                    nc.tensor.matmul(
                        ps_k[:, gs], wk8_sb[:, s],
                        x8_t[:, 2 * s:2 * s + 2, gs],
                        start=(s == 0), stop=(s == 3),
                        perf_mode=mybir.MatmulPerfMode.DoubleRow)
        nc.vector.tensor_scalar_add(kT_sb[:, tok], ps_k, b_sb[:, 1:2])

        ps_v = pp_pool.tile([128, JBLK], F32, tag="pp")
        for kt in range(KT):
            nc.tensor.matmul(ps_v, w_ap(2, kt), xsl(kt),
                             start=(kt == 0), stop=(kt == KT - 1))
        vT_t = vt_pool.tile([128, JBLK], FP16, tag="vt")
        nc.vector.tensor_scalar_add(vT_t, ps_v, b_sb[:, 2:3])
        for c in range(4):
            ps_tp = pp_pool.tile([128, 512], FP16, tag="pp")
            dst = ps_tp[:, 0:128]
            nc.tensor.transpose(dst, vT_t[:, c * 128:(c + 1) * 128], ident_f16)
            jt = jb * 4 + c
            nc.vector.tensor_copy(v_sb[:, jt * 128:(jt + 1) * 128], dst)

        if jb < 2:  # Q projection for the core's own tokens (rolled blocks 0/1)
            ps_q = pp_pool.tile([128, JBLK], F32, tag="pp")
            for kt in range(KT):
                nc.tensor.matmul(ps_q, w_ap(0, kt), xsl(kt),
                                 start=(kt == 0), stop=(kt == KT - 1))
            nc.scalar.activation(out=qT_sb[:, jb * JBLK:(jb + 1) * JBLK], in_=ps_q,
                                 func=mybir.ActivationFunctionType.Identity,
                                 bias=b_sb[:, 0:1], scale=1.0)

    # The V-matmuls run two j-tiles behind the S-matmuls (software
    # pipeline): V(jt-2) executes while ACT computes exp(jt-1)/exp(jt),
    # so the in-order PE never waits a full exp latency.
    pend = []

    def emit_v(jt, e):
        kj = slice(jt * 128, (jt + 1) * 128)
        for qb in range(NQB):
            qs = slice(qb * QBLK, (qb + 1) * QBLK)
            nc.tensor.matmul(po_t[:, qs], v_sb[:, kj], e[:, qs],
                             start=(jt == 0), stop=(jt == NJT - 1))

    def attention_block(jb):
        for c in range(4):
            jt = jb * 4 + c
            kj = slice(jt * 128, (jt + 1) * 128)
            ps_s = ps_pool.tile([128, NB], F32, tag="ps")
            for qb in range(NQB):
                qs = slice(qb * QBLK, (qb + 1) * QBLK)
                nc.tensor.matmul(ps_s[:, qs], kT_sb[:, kj], qT_sb[:, qs],
                                 start=True, stop=True)
            e = exp_pool.tile([128, NB], FP16, tag="exp")
            nc.scalar.activation(out=e, in_=ps_s,
                                 func=mybir.ActivationFunctionType.Exp,
                                 scale=SCALE_K)
            if jt == 0:
                nc.vector.tensor_copy(acc_a, e)
            elif jt == 32:
                nc.vector.tensor_copy(acc_b, e)
            elif jt < 32:
                nc.vector.tensor_add(acc_a, acc_a, e)
            else:
                nc.vector.tensor_add(acc_b, acc_b, e)
            if len(pend) >= 3:
                emit_v(*pend.pop(0))
            pend.append((jt, e))

    # --- PE warm-up -------------------------------------------------------
    # ~4us of dummy matmuls during the initial DMA wait flips the PE HAM
    # clock gate to 8/8 before the real work arrives (PE is idle anyway).
    warm = singles.tile([128, 512], BF16, tag="warm")
    nc.vector.memset(warm, 0.0)
    ps_w = ps_pool.tile([128, NB], F32, tag="ps")
    for _ in range(26):
        nc.tensor.matmul(ps_w[:, 0:512], warm[:, 0:128], warm,
                         start=True, stop=True)

    # --- main stream ------------------------------------------------------
    NG = NB // 128

    def den_half_a():
        # acc_a is final after attention_block(8) (jt 32-35 go to acc_b);
        # summing its partitions here overlaps the PE+DVE work with the
        # remaining attention stream instead of the serial tail.
        ps_da = ps_pool.tile([128, NB], F32, tag="ps")
        for g in range(NG):
            nc.tensor.matmul(ps_da[:, g:g + 1],
                             acc_a[:, g * 128:(g + 1) * 128], ones128,
                             start=True, stop=True)
        nc.vector.tensor_copy(den_ab, ps_da[:, 0:NG])

    stream_block(0)
    stream_block(1)
    attention_block(0)
    for jb in range(2, NJB):
        stream_block(jb)
        attention_block(jb - 1)
        if jb == 10:  # after attention_block(9): jt 36-39 done, acc_a final
            den_half_a()
    attention_block(NJB - 1)
    while pend:  # flush the lag-2 pipelined V-matmuls in order
        emit_v(*pend.pop(0))

    # --- epilogue ---------------------------------------------------------
    # denominator: acc_a's partition-sums already ran mid-stream; only
    # acc_b's ones-matmuls and the A+B combine remain on the tail.
    ps_d = ps_pool.tile([128, NB], F32, tag="ps")
    for g in range(NG):
        nc.tensor.matmul(ps_d[:, g:g + 1],
                         acc_b[:, g * 128:(g + 1) * 128], ones128,
                         start=True, stop=True)
    nc.vector.tensor_add(den_ab, den_ab, ps_d[:, 0:NG])
    nc.vector.reciprocal(rden_sb, den_ab)
    for g in range(NB // 128):
        # O^T -> SBUF on the (tail-idle) scalar engine so the PE's
        # transposes aren't queued behind DVE's reciprocal; scale by 1/den
        # on DVE, store.
        oT_t = oT_pool.tile([128, 128], BF16, tag="oT")
        nc.scalar.activation(out=oT_t, in_=po_t[:, g * 128:(g + 1) * 128],
                             func=mybir.ActivationFunctionType.Identity,
                             scale=1.0)
        ps_to = pp_pool.tile([128, 512], BF16, tag="pp")
        dst = ps_to[:, 0:128]
        nc.tensor.transpose(dst, oT_t, ident_bf)
        ob = o_pool.tile([128, DH], F32, tag="o")
        nc.vector.tensor_scalar_mul(ob, dst, rden_sb[:, g:g + 1])
        nc.sync.dma_start(out=out[g * 128:(g + 1) * 128, :], in_=ob)


def build_nc():
    if "nc" in _CACHE:
        return _CACHE["nc"]
    from contextlib import ExitStack

    nc = bacc.Bacc("TRN2", target_bir_lowering=False, debug=False,
                   num_devices=N_CORES)
    xT = nc.dram_tensor("xT", [NJB, 128, FB], BF16, kind="ExternalInput").ap()
    xT8 = nc.dram_tensor("xT8", [NJB, 128, KT, JBLK], FP8,
                         kind="ExternalInput").ap()
    wk8 = nc.dram_tensor("wk8", [128, 4, 2, 128], FP8,
                         kind="ExternalInput").ap()
    w_all = nc.dram_tensor("w_all", [128, 3 * D], BF16, kind="ExternalInput").ap()
    b_all = nc.dram_tensor("b_all", [128, 3], F32, kind="ExternalInput").ap()
    out = nc.dram_tensor("out", [NB, DH], F32, kind="ExternalOutput").ap()

    with tile.TileContext(nc) as tc:
        with ExitStack() as ctx:
            _emit(ctx, tc, nc, xT, xT8, wk8, w_all, b_all, out)
    nc.compile()
    _CACHE["nc"] = nc
    return nc


def make_in_maps(inputs):
    x = np.asarray(inputs["x"], dtype=np.float32)
    # blocked x.T: blk[jb, p, kt*JBLK + n] = x.T[kt*128 + p, jb*JBLK + n]
    #            = x[jb*JBLK + n, kt*128 + p]
    import ml_dtypes
    np_fp8 = mybir.dt.np(FP8)
    xb = x.reshape(NJB, JBLK, KT, 128)                    # [jb, n, kt, p]
    blkf = np.ascontiguousarray(
        xb.transpose(0, 3, 2, 1)).reshape(NJB, 128, FB)   # [jb, p, kt*n]
    blk = blkf.astype(ml_dtypes.bfloat16)
    blk8 = blkf.reshape(NJB, 128, KT, JBLK).astype(np_fp8)

    w_cols = []
    for wn in ("Wk", "Wv", "Wq"):
        w = np.asarray(inputs[wn], np.float32)            # [D, DH]
        if wn == "Wk":
            w = 16.0 * w  # match the fp8 path: kT_sb holds 16*K everywhere
        wr = w.reshape(KT, 128, DH).transpose(1, 0, 2).reshape(128, D)
        w_cols.append(wr)
    w_all = np.concatenate(w_cols, axis=1).astype(ml_dtypes.bfloat16)
    # fp8 Wk, pre-scaled by 16, packed for DoubleRow:
    # wk8[p, s, i, dk] = 16*Wk[256s + 128i + p, dk]
    wk8 = np.ascontiguousarray(
        (16.0 * np.asarray(inputs["Wk"], np.float32))
        .reshape(4, 2, 128, DH).transpose(2, 0, 1, 3)).astype(np_fp8)
    b_all = np.ascontiguousarray(np.stack(
        [np.asarray(inputs["bq"], np.float32),
         16.0 * np.asarray(inputs["bk"], np.float32),
         np.asarray(inputs["bv"], np.float32)],
        axis=1))                                          # [128, 3]

    in_maps = []
    for c in range(N_CORES):
        m = {
            "xT": np.ascontiguousarray(np.roll(blk, -2 * c, axis=0)),
            "xT8": np.ascontiguousarray(np.roll(blk8, -2 * c, axis=0)),
            "wk8": wk8,
            "w_all": w_all,
            "b_all": b_all,
        }
        in_maps.append(m)
    return in_maps


def kernel(**inputs) -> np.ndarray:
    global LAST_RESULTS
    nc = build_nc()
    in_maps = make_in_maps(inputs)
    res = run_bass_kernel_spmd(nc, in_maps, core_ids=list(range(N_CORES)))
    LAST_RESULTS = res
    return np.concatenate([res.results[c]["out"] for c in range(N_CORES)],
                          axis=0)

